# revision 8
# baseline (speedup 1.0000x reference)
"""KAN basis-linear kernel for 8 TRN2 NeuronCores.

Computes, for x:[B,I], spline_weight:[O,I,K=9], base_weight:[O,I], bias:[O]:

    basis = relu(1 - |(clip(x,-2,2)[...,None] - grid) / delta|)   # hat basis
    out   = einsum('bik,oik->bo', basis, spline_weight)
          + silu(x) @ base_weight.T + bias

Strategy: data-parallel over the batch across 8 cores (weights replicated).

Algebra (exact): with grid g_k = -2 + 0.5k, Abel summation over the hat
partition-of-unity gives
    sum_k hat_k(xc) * sw_k = sum_{j=0..7} psi_j(x) * (sw_j - sw_{j+1}) + sw_8
where psi_j(x) = clip(2*(g_{j+1} - x), 0, 1)  (ramp; saturation subsumes the
clip of x, so raw x is used). sw_8 folds into the bias. Together with the
silu base branch this is ONE 9-channel contraction:
    phi[b,i,ch] = [psi_0..psi_7, silu(x)],  W[ch,i,o] = [sw_j - sw_{j+1}, bw]
computed on-chip: ACT produces relu(2g_{j+1}-2x), DVE min(.,1) casts to bf16,
TensorEngine contracts (i,ch) in 72 chunks of 128 accumulating fp32 in PSUM
(8 banks = 8 o-tiles of [128o x 512b]); bias added during PSUM evacuation.
bf16 operands + fp32 accumulation measure ~5e-3 relative error vs the fp32
reference (validated off-line), well under the 2e-2 gate.
"""
import numpy as np
import ml_dtypes
from contextlib import ExitStack

import concourse.bass as bass
import concourse.tile as tile
import concourse.mybir as mybir
from concourse import bacc
from concourse.bass_utils import run_bass_kernel_spmd

N_CORES = 8
B, I, O, K = 16384, 1024, 1024, 9
B_CORE = B // N_CORES            # 2048 batch rows per core
B_SUPER = 512                    # batch stripe held in PSUM (1 bank per o-tile)
N_SUPERS = B_CORE // B_SUPER     # 4
P = 128
N_ICHK = I // P                  # 8 contraction chunks over i
N_CH = 9                         # 8 ramp channels + 1 silu channel
N_OT = O // P                    # 8 output tiles (one PSUM bank each)

F32 = mybir.dt.float32
BF16 = mybir.dt.bfloat16
AF = mybir.ActivationFunctionType
ALU = mybir.AluOpType

_CACHE = {}


def _build():
    nc = bacc.Bacc("TRN2", target_bir_lowering=False, debug=False,
                   num_devices=N_CORES)
    # x tiled on host: [bs, ichk, p, b]
    xt = nc.dram_tensor("xt", [N_SUPERS, N_ICHK, P, B_SUPER], F32,
                        kind="ExternalInput").ap()
    # weights tiled on host: [ichk, p, ch, o] (per-ichk slice is contiguous)
    w = nc.dram_tensor("w", [N_ICHK, P, N_CH, O], BF16,
                       kind="ExternalInput").ap()
    bias = nc.dram_tensor("bias", [O], F32, kind="ExternalInput").ap()
    # output tiled: [ot, bs, p, b] (contiguous 256KB stores)
    outT = nc.dram_tensor("outT", [N_OT, N_SUPERS, P, B_SUPER], F32,
                          kind="ExternalOutput").ap()

    with tile.TileContext(nc) as tc, ExitStack() as ctx:
        const_pool = ctx.enter_context(tc.tile_pool(name="const", bufs=1))
        x_pool = ctx.enter_context(tc.tile_pool(name="xin", bufs=3))
        t_pool = ctx.enter_context(tc.tile_pool(name="tmp", bufs=3))
        phi_pool = ctx.enter_context(tc.tile_pool(name="phi", bufs=N_ICHK))
        w_pool = ctx.enter_context(tc.tile_pool(name="wts", bufs=4))
        out_pool = ctx.enter_context(tc.tile_pool(name="outs", bufs=3))
        psum_pool = ctx.enter_context(
            tc.tile_pool(name="psum", bufs=N_OT, space="PSUM"))

        # ACT bias constants: 2*g_{j+1} = j - 3 for j=0..7
        consts = const_pool.tile([P, 8], F32)
        for j in range(8):
            nc.any.memset(consts[:, j:j + 1], float(j - 3))

        # bias[o] -> [128, 8] with o = ot*128 + p
        bias_sb = const_pool.tile([P, N_OT], F32)
        nc.scalar.dma_start(bias_sb[:], bias.rearrange("(ot p) -> p ot", p=P))

        # Small PE warm-up spin bridging the first input-DMA wait: starts
        # the HAM busy-streak early so the clock-gate reaches 8/8 sooner.
        warm = const_pool.tile([P, B_SUPER], BF16)
        nc.any.memset(warm[:], 0.0)
        warm_ps = psum_pool.tile([P, B_SUPER], F32, tag="psum")
        for _ in range(4):
            nc.tensor.matmul(warm_ps[:], lhsT=warm[:, :P], rhs=warm[:],
                             start=True, stop=True)

        for bs in range(N_SUPERS):
            # ---- phi production (ACT relu-ramp + DVE min/cast + ACT silu) ----
            phis = []
            for ichk in range(N_ICHK):
                x_sb = x_pool.tile([P, B_SUPER], F32, tag="xin")
                nc.scalar.dma_start(x_sb[:], xt[bs, ichk])
                phi = phi_pool.tile([P, N_CH, B_SUPER], BF16, tag="phi")
                for j in range(8):
                    # t = relu(2*g_{j+1} - 2*x) ; psi_j = min(t, 1)
                    t = t_pool.tile([P, B_SUPER], F32, tag="tmp")
                    nc.scalar.activation(t[:], x_sb[:], AF.Relu,
                                         bias=consts[:, j:j + 1], scale=-2.0)
                    nc.vector.tensor_scalar_min(phi[:, j, :], t[:], 1.0)
                # silu on raw x
                nc.scalar.activation(phi[:, 8, :], x_sb[:], AF.Silu)
                phis.append(phi)

            # ---- matmuls: contract over (i, ch) in 72 chunks of 128 ----
            psums = [psum_pool.tile([P, B_SUPER], F32, tag="psum",
                                    name=f"psum_{bs}_{ot}")
                     for ot in range(N_OT)]
            for ichk in range(N_ICHK):
                w_sb = w_pool.tile([P, N_CH, O], BF16, tag="wts")
                if bs == 0 and ichk == 0:
                    # per-channel DMAs: first matmul starts after ~250KB
                    for c0 in range(N_CH):
                        nc.sync.dma_start(w_sb[:, c0:c0 + 1, :],
                                          w[ichk, :, c0:c0 + 1, :])
                else:
                    # one big transfer amortizes the ~2us DMA completion
                    # latency on the serial HWDGE queue
                    nc.sync.dma_start(w_sb[:], w[ichk])
                # ch-major on the very first chunk (matmuls start after one
                # psi channel); ot-major elsewhere so each PSUM bank's
                # last/first touch is staggered and evacuation overlaps MMs.
                if bs == 0 and ichk == 0:
                    order = [(ch, ot) for ch in range(N_CH)
                             for ot in range(N_OT)]
                else:
                    order = [(ch, ot) for ot in range(N_OT)
                             for ch in range(N_CH)]
                for ch, ot in order:
                    nc.tensor.matmul(
                        psums[ot][:],
                        lhsT=w_sb[:, ch, bass.ts(ot, P)],
                        rhs=phis[ichk][:, ch, :],
                        start=(ichk == 0 and ch == 0),
                        stop=(ichk == N_ICHK - 1 and ch == N_CH - 1),
                    )

            # ---- evacuate PSUM + bias add (DVE), DMA out ----
            for ot in range(N_OT):
                o_sb = out_pool.tile([P, B_SUPER], F32, tag="outs")
                nc.vector.tensor_scalar_add(o_sb[:], psums[ot][:],
                                            bias_sb[:, ot:ot + 1])
                nc.scalar.dma_start(outT[ot, bs], o_sb[:])

    nc.compile()
    return nc


def _get_nc():
    if "nc" not in _CACHE:
        _CACHE["nc"] = _build()
    return _CACHE["nc"]


def _stage_inputs(x, spline_weight, base_weight, bias):
    """Host-side input staging shared by kernel() and test harnesses."""
    # x[b, i] -> [core, bs, ichk, p, b_super]
    xt = np.ascontiguousarray(
        x.reshape(N_CORES, N_SUPERS, B_SUPER, N_ICHK, P)
        .transpose(0, 1, 3, 4, 2))
    # W[ch, i, o]: ramp diffs + base weight -> [ichk, p, ch, o] bf16
    rho = spline_weight[..., :8] - spline_weight[..., 1:]   # [O, I, 8]
    w_full = np.concatenate([rho, base_weight[..., None]], axis=2)  # [O, I, 9]
    w_dev = np.ascontiguousarray(
        w_full.transpose(1, 2, 0)                            # [I, 9, O]
        .reshape(N_ICHK, P, N_CH, O).astype(ml_dtypes.bfloat16))
    # bias fold: bias + sum_i sw[o, i, 8]
    bias_dev = (bias + spline_weight[..., 8].sum(axis=1)).astype(np.float32)
    return xt, w_dev, bias_dev


def kernel(x, spline_weight, base_weight, bias):
    x = np.asarray(x, dtype=np.float32)
    spline_weight = np.asarray(spline_weight, dtype=np.float32)
    base_weight = np.asarray(base_weight, dtype=np.float32)
    bias = np.asarray(bias, dtype=np.float32)

    nc = _get_nc()
    xt, w_dev, bias_dev = _stage_inputs(x, spline_weight, base_weight, bias)

    in_maps = [{"xt": np.ascontiguousarray(xt[c]), "w": w_dev,
                "bias": bias_dev} for c in range(N_CORES)]
    res = run_bass_kernel_spmd(nc, in_maps, core_ids=list(range(N_CORES)))

    # outT[ot, bs, p, b] per core -> out[b, o]
    outs = []
    for c in range(N_CORES):
        oc = np.asarray(res.results[c]["outT"])
        outs.append(oc.transpose(1, 3, 0, 2).reshape(B_CORE, O))
    return np.ascontiguousarray(np.concatenate(outs, axis=0),
                                dtype=np.float32)
